# revision 43
# baseline (speedup 1.0000x reference)
"""Distributed Trainium2 Bass kernel for ArcticAttention (GQA + RoPE + sliding window).

Sharding: tensor-parallel over heads across 8 cores. Core c owns q heads
4c..4c+3 and kv head c (exactly one GQA group). Per core:
  - q/k projections (bf16 matmuls, fp32 PSUM) producing qT/kT [dh, tok],
  - v projection computed directly in [tok, dh] layout (hidden chunk as the
    stationary operand) so no PE transposes are needed,
  - RoPE fused on the vector engine from host-precomputed cos/sign-folded-sin
    tables,
  - sliding-window attention in S^T = K@Q^T layout (softmax over the
    partition axis via a ones-vector matmul; 1/l broadcast via gpsimd
    partition_broadcast; PV matmul needs no transposes anywhere),
  - per-token-block (512) AllGather of ctx^T features (bf16, 512KB/core),
  - column-sharded o_proj producing out^T [oc, tok]; host concatenates.

Scheduling: the attention tile loop is Scalar-EXP paced (~1.6ns/col vs PE
0.5ns/col), so o_proj matmul chains for block i-2 are woven between
attention tiles of block i to keep the PE dense; the per-block AllGathers
make that input available two blocks after each attention block finishes.
"""

import sys

sys.path.insert(0, "/opt/pypackages")
sys.path.insert(0, "/opt/trn_rl_repo")

import numpy as np
import ml_dtypes

BF16 = ml_dtypes.bfloat16

B, S, HID = 2, 2048, 4096
H, HKV, DH = 32, 8, 128
G = H // HKV
WIN = 1024
THETA = 10000.0
NCORES = 8
HPC = H // NCORES          # 4 q heads per core
BT = B * S                 # 4096 tokens
QB = 512                   # token block for projections, attention, o_proj
NQB = S // QB              # 4 blocks per batch
NA = HID // 128            # 32 hid chunks
NQTR = NA // 4             # 8 a-chunks per cf quarter tile
SCALE = 1.0 / float(np.sqrt(DH))
NBLK = B * NQB             # 8 token blocks total

MASK_RS = (0, 1, 2, 3, 8, 9, 10, 11)
MSLOT = {r: i for i, r in enumerate(MASK_RS)}

# Steady-state blocks use 1MB-class DMAs (each hardware DMA queue sustains
# ~47GB/s; few large DMAs across queues win). Block 0 is different: the PE
# consumes wq+hid chunk a at ~t0+0.78a us, so its segments are sized to land
# just-in-time when spread over parallel queues — a single 1MB segment
# (21us on one queue) arrives ~10us after its chunks are needed.
SEG_COARSE = [(0, 8), (8, 8), (16, 8), (24, 8)]
# wq tiles are persistent (cpool) so wq can split 10 ways; hid block-0 segs
# share the 8-slot hid pool with the hid-1 prefetch, so 8 ways max.
SEG_FIRST = [(0, 1), (1, 1), (2, 3), (5, 3), (8, 4), (12, 4), (16, 4),
             (20, 4), (24, 4), (28, 4)]
SEG_HID0 = [(0, 1), (1, 1), (2, 3), (5, 3), (8, 4), (12, 4), (16, 8),
            (24, 8)]


def _span(r):
    qlo = max(0, (r - 8) * 128)
    qhi = min(QB, (r - 8) * 128 + 1024 + 127)
    return qlo, qhi


def _build_nc():
    import concourse.bass as bass
    import concourse.bacc as bacc
    import concourse.mybir as mybir
    from concourse import tile

    dt = mybir.dt
    bf = dt.bfloat16
    f32 = dt.float32
    AF = mybir.ActivationFunctionType

    nc = bacc.Bacc(
        "TRN2",
        target_bir_lowering=False,
        debug=False,
        enable_asserts=False,
        num_devices=NCORES,
    )

    # hidden, pre-tiled on host as [tb, p, a, t]: per token-block each SBUF
    # partition's data (all 32 a-chunks) is one contiguous 32KB DRAM run, so
    # the loads stream at full rate with a plain 2D access pattern.
    hiddenT = nc.dram_tensor("hiddenT", [NBLK * 128, NA * QB], bf,
                             kind="ExternalInput")
    # weights/masks host-prearranged to [128, chunks*width] matching the SBUF
    # tile layout exactly -> plain contiguous 2D DMAs
    wq = nc.dram_tensor("wq", [128, NA * HPC * DH], bf, kind="ExternalInput")
    wk = nc.dram_tensor("wk", [128, NA * DH], bf, kind="ExternalInput")
    wv = nc.dram_tensor("wv", [128, NA * DH], bf, kind="ExternalInput")
    wo = nc.dram_tensor("wo", [128, NA * HPC * DH], bf, kind="ExternalInput")
    cost = nc.dram_tensor("cost", [DH, S], bf, kind="ExternalInput")
    sinm = nc.dram_tensor("sinm", [DH, S], bf, kind="ExternalInput")
    maskt = nc.dram_tensor("maskt", [128, len(MASK_RS) * QB], bf, kind="ExternalInput")
    outT = nc.dram_tensor("outT", [HPC * DH, BT], f32, kind="ExternalOutput")

    with tile.TileContext(nc) as tc:
        with (
            tc.tile_pool(name="const", bufs=1) as cpool,
            tc.tile_pool(name="hid", bufs=8) as hidpool,
            tc.tile_pool(name="kv", bufs=2) as kvpool,
            tc.tile_pool(name="qt", bufs=8) as qtpool,
            tc.tile_pool(name="work", bufs=2) as wpool,
            tc.tile_pool(name="pt", bufs=4) as ptpool,
            tc.tile_pool(name="mm", bufs=4, space="PSUM") as mmpool,
            tc.tile_pool(name="sps", bufs=2, space="PSUM") as spool,
            tc.tile_pool(name="ctxps", bufs=2, space="PSUM") as cxpool,
            tc.tile_pool(name="dram", bufs=1, space="DRAM") as dpool,
        ):
            # ---- resident constants ----
            wq_sbs = [
                cpool.tile([128, w * HPC * DH], bf, tag=f"wq{i}", name=f"wq{i}")
                for i, (a0, w) in enumerate(SEG_FIRST)
            ]
            _wq_seg_of = {}
            for i, (a0, w) in enumerate(SEG_FIRST):
                for a in range(a0, a0 + w):
                    _wq_seg_of[a] = (i, a - a0)

            def wq_slice(a, h):
                i, ao = _wq_seg_of[a]
                return wq_sbs[i][:, ao * 512 + h * 128 : ao * 512 + (h + 1) * 128]

            wk_sb = cpool.tile([128, NA * DH], bf, tag="wk")
            wv_sb = cpool.tile([128, NA * DH], bf, tag="wv")
            wo_sb = cpool.tile([128, NA * HPC * DH], bf, tag="wo")
            cos_sb = cpool.tile([128, S], bf, tag="cos")
            sin_sb = cpool.tile([128, S], bf, tag="sin")
            mask_sb = cpool.tile([128, len(MASK_RS) * QB], bf, tag="mask")
            ones_sb = cpool.tile([128, 1], bf, tag="ones")

            # Startup preloads: few big DMAs, consumption-ordered. wo (4MB,
            # first needed ~200us in) is deferred to block 1 so it doesn't
            # compete for queue bandwidth during the DMA-bound first block.
            def wq_dma(eng, i):
                a0, w = SEG_FIRST[i]
                eng.dma_start(wq_sbs[i][:], wq[:, a0 * 512 : (a0 + w) * 512])

            for i in range(4):
                wq_dma(nc.scalar, i)
            nc.scalar.dma_start(cos_sb[:], cost[:])
            nc.scalar.dma_start(sin_sb[:], sinm[:])
            for i in range(4, len(SEG_FIRST)):
                wq_dma(nc.gpsimd, i)
            nc.gpsimd.dma_start(wk_sb[:], wk[:])
            nc.gpsimd.dma_start(wv_sb[:], wv[:])
            nc.gpsimd.dma_start(mask_sb[:], maskt[:])
            nc.any.memset(ones_sb[:], 1.0)

            def load_wo():
                for i in range(4):
                    nc.gpsimd.dma_start(
                        wo_sb[:, i * 4096 : (i + 1) * 4096],
                        wo[:, i * 4096 : (i + 1) * 4096],
                    )

            # per token-block collective buffers; block 7's is split by head
            # pair so its first AllGather can start mid-attention (the tail
            # o_proj would otherwise wait on the full last AllGather)
            ctxl = [
                dpool.tile([HPC * DH, QB], bf, tag=f"ctxl{k}", name=f"ctxl{k}")
                for k in range(NBLK - 1)
            ]
            ctxf = [
                dpool.tile([H * DH, QB], bf, addr_space="Shared",
                           tag=f"ctxf{k}", name=f"ctxf{k}")
                for k in range(NBLK - 1)
            ]
            ctxl7 = [
                dpool.tile([2 * DH, QB], bf, tag=f"ctxl7{i}", name=f"ctxl7{i}")
                for i in range(2)
            ]
            ctxf7 = [
                dpool.tile([NCORES * 2 * DH, QB], bf, addr_space="Shared",
                           tag=f"ctxf7{i}", name=f"ctxf7{i}")
                for i in range(2)
            ]
            # dummy collective to absorb the ~12us first-collective ramp
            # before the first real AllGather needs it
            duml = dpool.tile([8, 16], bf, tag="duml", name="duml")
            dumf = dpool.tile([64, 16], bf, addr_space="Shared", tag="dumf",
                              name="dumf")
            dums = cpool.tile([8, 16], bf, tag="dums")
            nc.any.memset(dums[:], 0.0)
            nc.gpsimd.dma_start(duml[:], dums[:])
            nc.gpsimd.collective_compute(
                "AllGather",
                mybir.AluOpType.bypass,
                replica_groups=[list(range(NCORES))],
                ins=[duml[:].opt()],
                outs=[dumf[:].opt()],
            )

            def load_hid_segs(k):
                segs = SEG_HID0 if k == 0 else SEG_COARSE
                out = []
                for a0, w in segs:
                    t = hidpool.tile([128, w * QB], bf, tag="hid", name=f"hid{k}_{a0}")
                    nc.sync.dma_start(
                        t[:],
                        hiddenT[k * 128 : (k + 1) * 128,
                                a0 * QB : (a0 + w) * QB],
                    )
                    out.append((a0, w, t))
                return out

            def rope_drain(ps, dst, qbi):
                """dst(bf16) = ps * cos + rot_half(ps) * sin (sign-folded)."""
                t1 = wpool.tile([128, QB], f32, tag="ropet1", bufs=1)
                t2 = wpool.tile([128, QB], f32, tag="ropet2", bufs=1)
                tok0 = qbi * QB
                cs = cos_sb[:, tok0 : tok0 + QB]
                sn = sin_sb[:, tok0 : tok0 + QB]
                nc.vector.tensor_mul(t1[:], ps, cs)
                nc.vector.tensor_mul(t2[0:64, :], ps[64:128, :], sn[0:64, :])
                nc.vector.tensor_mul(t2[64:128, :], ps[0:64, :], sn[64:128, :])
                nc.vector.tensor_add(dst, t1[:], t2[:])

            def v_fillers(k, qbi, v_sb, hsegs):
                """Direct v[tok, dh] projection for block k as filler pieces:
                hidden chunk as stationary, wv as moving. Each 128-token
                subtile accumulates in its own PSUM tile."""
                ltok = qbi * QB
                hs_of = {}
                for a0, w, hs in hsegs:
                    for ai in range(w):
                        hs_of[a0 + ai] = hs[:, ai * QB : (ai + 1) * QB]
                out = []
                state = {}

                def mk_chunk(j, a0):
                    def emit():
                        if j not in state:
                            state[j] = mmpool.tile(
                                [128, 128], f32, tag="mmps", name=f"vps{k}_{j}"
                            )
                        vps = state[j]
                        for a in range(a0, a0 + 8):
                            nc.tensor.matmul(
                                vps[:],
                                hs_of[a][:, j * 128 : (j + 1) * 128],
                                wv_sb[:, a * 128 : (a + 1) * 128],
                                start=(a == 0), stop=(a == NA - 1),
                            )
                    return emit

                def mk_drain(j):
                    def emit():
                        vps = state.pop(j)
                        nc.vector.tensor_copy(
                            v_sb[:, ltok + j * 128 : ltok + (j + 1) * 128], vps[:]
                        )
                    return emit

                for j in range(4):
                    for a0 in range(0, NA, 8):
                        out.append(mk_chunk(j, a0))
                    out.append(mk_drain(j))
                return out

            def proj_block(k, b, qbi, kT_sb, v_sb, hsegs, with_v):
                """q/k projections + RoPE (+ inline direct-v for early blocks)."""
                ltok = qbi * QB
                qts = [
                    qtpool.tile([128, QB], bf, tag="qtile", name=f"qt{k}_{h}")
                    for h in range(HPC)
                ]
                for grp in (("q0", "q1", "q2"), ("q3", "k")):
                    ps = {u: mmpool.tile([128, QB], f32, tag="mmps", name=f"ps{u}{k}")
                          for u in grp}
                    for a0, w, hs in hsegs:
                        for u in grp:
                            for ai in range(w):
                                a = a0 + ai
                                st = a == 0
                                sp = a == NA - 1
                                hcol = hs[:, ai * QB : (ai + 1) * QB]
                                if u[0] == "q":
                                    h = int(u[1])
                                    nc.tensor.matmul(
                                        ps[u][:], wq_slice(a, h), hcol,
                                        start=st, stop=sp,
                                    )
                                else:
                                    nc.tensor.matmul(
                                        ps[u][:],
                                        wk_sb[:, a * 128 : (a + 1) * 128],
                                        hcol,
                                        start=st, stop=sp,
                                    )
                    for u in grp:
                        if u[0] == "q":
                            rope_drain(ps[u][:], qts[int(u[1])][:], qbi)
                        else:
                            rope_drain(ps[u][:], kT_sb[:, ltok : ltok + QB], qbi)
                if with_v:
                    for f in v_fillers(k, qbi, v_sb, hsegs):
                        f()
                return qts

            class OProjChain:
                """o_proj for ctx block `opk`, emitted as small filler pieces."""

                def __init__(self, opk):
                    self.opk = opk
                    self.cfs = None

                def prefetch(self):
                    self.cfs = []
                    if self.opk < NBLK - 1:
                        src3 = ctxf[self.opk][:].rearrange("(a p) t -> p a t", p=128)
                        for qt4 in range(4):
                            t = hidpool.tile(
                                [128, NQTR * QB], bf, tag="hid",
                                name=f"cf{self.opk}_{qt4}",
                            )
                            nc.gpsimd.dma_start(
                                t[:].rearrange("p (a t) -> p a t", a=NQTR),
                                src3[:, qt4 * NQTR : (qt4 + 1) * NQTR, :],
                            )
                            self.cfs.append(t)
                    else:
                        # block 7's ctx is AllGathered in two head-pair halves:
                        # global chunk a = 4c + q lives in half q//2 at rows
                        # c*256 + (q%2)*128
                        srcs = [
                            ctxf7[i][:].rearrange("(cq p) t -> p cq t", p=128)
                            for i in range(2)
                        ]
                        for qt4 in range(4):
                            t = hidpool.tile(
                                [128, NQTR * QB], bf, tag="hid",
                                name=f"cf{self.opk}_{qt4}",
                            )
                            dst3 = t[:].rearrange("p (a t) -> p a t", a=NQTR)
                            for cr in range(2):
                                c = qt4 * 2 + cr
                                for i in range(2):
                                    nc.gpsimd.dma_start(
                                        dst3[:, cr * 4 + i * 2 : cr * 4 + i * 2 + 2, :],
                                        srcs[i][:, c * 2 : c * 2 + 2, :],
                                    )
                            self.cfs.append(t)

                def fillers(self):
                    out = []
                    state = {}

                    def mk_chunk(oc, a0):
                        def emit():
                            if oc not in state:
                                state[oc] = mmpool.tile(
                                    [128, QB], f32, tag="mmps",
                                    name=f"ops{self.opk}_{oc}",
                                )
                            ps = state[oc]
                            for a in range(a0, a0 + 4):
                                nc.tensor.matmul(
                                    ps[:],
                                    wo_sb[:, a * 512 + oc * 128 : a * 512 + (oc + 1) * 128],
                                    self.cfs[a // NQTR][:, (a % NQTR) * QB : (a % NQTR + 1) * QB],
                                    start=(a == 0), stop=(a == NA - 1),
                                )
                        return emit

                    def mk_drain(oc):
                        def emit():
                            ps = state.pop(oc)
                            osb = wpool.tile([128, QB], f32, tag="osb",
                                             name=f"ob{self.opk}_{oc}", bufs=1)
                            nc.vector.tensor_copy(osb[:], ps[:])
                            nc.sync.dma_start(
                                outT[oc * 128 : (oc + 1) * 128,
                                     self.opk * QB : (self.opk + 1) * QB],
                                osb[:],
                            )
                        return emit

                    for oc in range(HPC):
                        for a0 in range(0, NA, 4):
                            out.append(mk_chunk(oc, a0))
                        out.append(mk_drain(oc))
                    return out

            def attn_block(k, b, qbi, qts, kT_sb, v_sb, fillers):
                Q0 = 4 * qbi
                kts = [Q0] + [kt for kt in range(max(0, Q0 - 8), Q0 + 4) if kt != Q0]
                nslots = HPC * (len(kts) + 1)
                nf = len(fillers)
                fi = 0
                slot = 0

                def tick():
                    nonlocal fi, slot
                    slot += 1
                    want = nf * slot // nslots
                    while fi < want:
                        fillers[fi]()
                        fi += 1

                for h in range(HPC):
                    qt = qts[h]
                    ctx_ps = cxpool.tile([128, QB], f32, tag="ctxps", name=f"cx{k}{h}")
                    l_ps = spool.tile([1, QB], f32, tag="sps", name=f"l{k}{h}")
                    # bf16 accumulator for the prob-sum: 2x DVE rate and feeds
                    # the partition-sum ones-matmul directly
                    acc = wpool.tile([128, QB], bf, tag="lacc", name=f"la{k}{h}")
                    for idx, kt in enumerate(kts):
                        r = kt - (Q0 - 8)
                        qlo, qhi = _span(r)
                        s_ps = spool.tile([128, QB], f32, tag="sps", name=f"s{k}{h}{kt}")
                        nc.tensor.matmul(
                            s_ps[:, qlo:qhi],
                            kT_sb[:, kt * 128 : (kt + 1) * 128],
                            qt[:, qlo:qhi],
                            start=True, stop=True,
                        )
                        pt = ptpool.tile([128, QB], bf, tag="pt", name=f"pt{k}{h}{kt}")
                        nc.scalar.activation(
                            pt[:, qlo:qhi], s_ps[:, qlo:qhi], AF.Exp, scale=SCALE
                        )
                        if r in MSLOT:
                            m0 = MSLOT[r] * QB
                            nc.vector.tensor_mul(
                                pt[:, qlo:qhi],
                                pt[:, qlo:qhi],
                                mask_sb[:, m0 + qlo : m0 + qhi],
                            )
                        last = idx == len(kts) - 1
                        nc.tensor.matmul(
                            ctx_ps[:, qlo:qhi],
                            v_sb[:, kt * 128 : (kt + 1) * 128],
                            pt[:, qlo:qhi],
                            start=(idx == 0), stop=last,
                        )
                        if idx == 0:
                            nc.vector.tensor_copy(acc[:], pt[:])
                        else:
                            nc.vector.tensor_add(
                                acc[:, qlo:qhi], acc[:, qlo:qhi], pt[:, qlo:qhi]
                            )
                        tick()
                    nc.tensor.matmul(
                        l_ps[0:1, :], ones_sb[:, 0:1], acc[:], start=True, stop=True
                    )
                    lrec = wpool.tile([1, QB], f32, tag="lrec", name=f"lr{k}{h}", bufs=1)
                    nc.vector.reciprocal_approx_fast(lrec[:], l_ps[:])
                    lb = wpool.tile([128, QB], f32, tag="lb", name=f"lb{k}{h}")
                    nc.gpsimd.partition_broadcast(lb[:], lrec[0:1, :])
                    ctx_sb = wpool.tile([128, QB], bf, tag="ctxsb", name=f"cs{k}{h}")
                    nc.vector.tensor_mul(ctx_sb[:], ctx_ps[:], lb[:])
                    if k == NBLK - 1:
                        nc.sync.dma_start(
                            ctxl7[h // 2][(h % 2) * 128 : (h % 2 + 1) * 128, :],
                            ctx_sb[:],
                        )
                        if h % 2 == 1:
                            _ag(ctxl7[h // 2], ctxf7[h // 2])
                    else:
                        nc.sync.dma_start(
                            ctxl[k][h * 128 : (h + 1) * 128, :], ctx_sb[:]
                        )
                    tick()
                while fi < nf:
                    fillers[fi]()
                    fi += 1

            def _ag(lt, ft):
                nc.gpsimd.collective_compute(
                    "AllGather",
                    mybir.AluOpType.bypass,
                    replica_groups=[list(range(NCORES))],
                    ins=[lt[:].opt()],
                    outs=[ft[:].opt()],
                )

            def allgather(k):
                _ag(ctxl[k], ctxf[k])

            # ================= emission schedule =================
            chains = [OProjChain(opk) for opk in range(NBLK)]
            kT = {}
            vS = {}
            for b in range(B):
                kT[b] = kvpool.tile([128, S], bf, tag="kT", name=f"kT{b}")
                vS[b] = kvpool.tile([128, S], bf, tag="v", name=f"v{b}")

            hseg_of = {0: load_hid_segs(0)}
            for k in range(NBLK):
                b, qbi = divmod(k, NQB)
                # from block 2 on, prefetch next hidden at block top so the
                # next block's v-projection can be woven into this attention
                if k >= 2 and k + 1 < NBLK:
                    hseg_of[k + 1] = load_hid_segs(k + 1)
                qts = proj_block(k, b, qbi, kT[b], vS[b], hseg_of[k],
                                 with_v=(k <= 2))
                hseg_of.pop(k)
                if k == 1:
                    load_wo()
                # cf prefetch: the DMA only fires once proj(k) releases its
                # hid-pool slots, so cf(k-2) is ready one block later — the
                # weave map below lags by 3 blocks to match. cf5/cf6 are NOT
                # prefetched at block-7 top: their slot-WAR-blocked triggers
                # would sit at the head of the gpsimd queue and stall the
                # attention-7 epilogue broadcasts (delaying the last
                # AllGather by ~15us).
                if 2 <= k <= 6:
                    chains[k - 2].prefetch()
                if k == 7:
                    chains[6].prefetch()
                # early blocks: next hidden prefetched here instead — ahead of
                # this block's ctx stores in the sync queue, behind startup
                if k < 2:
                    hseg_of[k + 1] = load_hid_segs(k + 1)
                fillers = []
                if 3 <= k <= 6:
                    fillers += chains[k - 3].fillers()
                elif k == 7:
                    fillers += chains[4].fillers()
                if 2 <= k <= 6:
                    # v of the next block last: its hidden is still streaming
                    # in during this attention phase
                    nb, nq = divmod(k + 1, NQB)
                    fillers += v_fillers(k + 1, nq, vS[nb], hseg_of[k + 1])
                attn_block(k, b, qbi, qts, kT[b], vS[b], fillers)
                if k < NBLK - 1:
                    allgather(k)
            # tail: op6 -> op5 -> op7. op6 (input long ready) covers cf5's
            # load; op5 covers the last AllGather + cf7 so op7's input is
            # ready by the time the PE reaches it.
            chains[5].prefetch()
            for f in chains[6].fillers():
                f()
            for f in chains[5].fillers():
                f()
            chains[7].prefetch()
            for f in chains[7].fillers():
                f()

    nc.compile()
    return nc


_NC = None


def _get_nc():
    global _NC
    if _NC is None:
        _NC = _build_nc()
    return _NC


def _prep_inputs(hidden_states, q_proj_w, k_proj_w, v_proj_w, o_proj_w, position_ids):
    hidden_states = np.asarray(hidden_states, dtype=np.float32)
    # pre-tile: hT[tb, p, a, t] = hidden[tb*QB + t, a*128 + p]
    hT = np.ascontiguousarray(
        hidden_states.reshape(BT // QB, QB, NA, 128).transpose(0, 3, 2, 1)
    ).astype(BF16).reshape((BT // QB) * 128, NA * QB)

    pos = np.asarray(position_ids)[0].astype(np.float32)  # [S]
    inv = 1.0 / (THETA ** (np.arange(0, DH, 2, dtype=np.float32) / DH))  # [64]
    ang = pos[:, None] * inv[None, :]  # [S, 64]
    c = np.cos(ang).T.astype(np.float32)  # [64, S]
    s = np.sin(ang).T.astype(np.float32)
    cost = np.ascontiguousarray(np.concatenate([c, c], axis=0)).astype(BF16)
    sinm = np.ascontiguousarray(np.concatenate([-s, s], axis=0)).astype(BF16)

    kj = np.arange(128)[:, None]
    qi = np.arange(QB)[None, :]
    masks = []
    for r in MASK_RS:
        d = (8 - r) * 128 + qi - kj
        masks.append(((d >= 0) & (d < WIN)).astype(np.float32))
    maskt = np.ascontiguousarray(np.concatenate(masks, axis=0)).astype(BF16)

    q_proj_w = np.asarray(q_proj_w, dtype=np.float32)
    k_proj_w = np.asarray(k_proj_w, dtype=np.float32)
    v_proj_w = np.asarray(v_proj_w, dtype=np.float32)
    o_proj_w = np.asarray(o_proj_w, dtype=np.float32)

    def wtile(wT):
        """[HID, D] (hid-major) -> [128, NA*D] matching SBUF layout:
        out[p, a*D+dd] = wT[a*128+p, dd]."""
        dcols = wT.shape[1]
        return np.ascontiguousarray(
            wT.reshape(NA, 128, dcols).transpose(1, 0, 2).reshape(128, NA * dcols)
        ).astype(BF16)

    # maskt: [128, m*QB] with slot m at cols [m*QB, (m+1)*QB)
    maskt = np.ascontiguousarray(
        maskt.reshape(len(MASK_RS), 128, QB).transpose(1, 0, 2).reshape(
            128, len(MASK_RS) * QB
        )
    )

    in_maps = []
    for core in range(NCORES):
        r0q = core * HPC * DH
        r0k = core * DH
        in_maps.append(
            {
                "hiddenT": hT,
                "wq": wtile(q_proj_w[r0q : r0q + HPC * DH, :].T),
                "wk": wtile(k_proj_w[r0k : r0k + DH, :].T),
                "wv": wtile(v_proj_w[r0k : r0k + DH, :].T),
                "wo": wtile(o_proj_w[r0q : r0q + HPC * DH, :].T),
                "cost": cost,
                "sinm": sinm,
                "maskt": maskt,
            }
        )
    return in_maps


def run(inputs, trace=False):
    from concourse.bass_utils import run_bass_kernel_spmd

    nc = _get_nc()
    in_maps = _prep_inputs(
        inputs["hidden_states"],
        inputs["q_proj_w"],
        inputs["k_proj_w"],
        inputs["v_proj_w"],
        inputs["o_proj_w"],
        inputs["position_ids"],
    )
    res = run_bass_kernel_spmd(
        nc, in_maps, core_ids=list(range(NCORES)), trace=trace
    )
    out = np.empty((BT, HID), dtype=np.float32)
    for core in range(NCORES):
        o = np.asarray(res.results[core]["outT"], dtype=np.float32)  # [512, BT]
        out[:, core * HPC * DH : (core + 1) * HPC * DH] = o.T
    return out.reshape(B, S, HID), res


def kernel(**inputs):
    out, _ = run(inputs, trace=False)
    return out


# revision 48
# speedup vs baseline: 1.0083x; 1.0083x over previous
"""Distributed Trainium2 Bass kernel for ArcticAttention (GQA + RoPE + sliding window).

Sharding: tensor-parallel over heads across 8 cores. Core c owns q heads
4c..4c+3 and kv head c (exactly one GQA group). Per core:
  - q/k projections (bf16 matmuls, fp32 PSUM) producing qT/kT [dh, tok],
  - v projection computed directly in [tok, dh] layout (hidden chunk as the
    stationary operand) so no PE transposes are needed,
  - RoPE fused on the vector engine from host-precomputed cos/sign-folded-sin
    tables,
  - sliding-window attention in S^T = K@Q^T layout (softmax over the
    partition axis via a ones-vector matmul; 1/l broadcast via gpsimd
    partition_broadcast; PV matmul needs no transposes anywhere),
  - per-token-block (512) AllGather of ctx^T features (bf16, 512KB/core),
  - column-sharded o_proj producing out^T [oc, tok]; host concatenates.

Scheduling: the attention tile loop is Scalar-EXP paced (~1.6ns/col vs PE
0.5ns/col), so o_proj matmul chains for block i-2 are woven between
attention tiles of block i to keep the PE dense; the per-block AllGathers
make that input available two blocks after each attention block finishes.
"""

import sys

sys.path.insert(0, "/opt/pypackages")
sys.path.insert(0, "/opt/trn_rl_repo")

import numpy as np
import ml_dtypes

BF16 = ml_dtypes.bfloat16

B, S, HID = 2, 2048, 4096
H, HKV, DH = 32, 8, 128
G = H // HKV
WIN = 1024
THETA = 10000.0
NCORES = 8
HPC = H // NCORES          # 4 q heads per core
BT = B * S                 # 4096 tokens
QB = 512                   # token block for projections, attention, o_proj
NQB = S // QB              # 4 blocks per batch
NA = HID // 128            # 32 hid chunks
NQTR = NA // 4             # 8 a-chunks per cf quarter tile
SCALE = 1.0 / float(np.sqrt(DH))
NBLK = B * NQB             # 8 token blocks total

MASK_RS = (0, 1, 2, 3, 8, 9, 10, 11)
MSLOT = {r: i for i, r in enumerate(MASK_RS)}

# 1MB-class DMAs only: the hardware DMA queues sustain ~47GB/s each, so
# supply bandwidth comes from spreading few, large DMAs across queues —
# fine-grained splits serialize and starve the PE at startup. The one
# exception: the very first wq/hid pieces are small so the first matmul
# gates on ~512KB.
SEG_COARSE = [(0, 8), (8, 8), (16, 8), (24, 8)]
SEG_FIRST = [(0, 2), (2, 6), (8, 8), (16, 8), (24, 8)]


def _span(r):
    qlo = max(0, (r - 8) * 128)
    qhi = min(QB, (r - 8) * 128 + 1024 + 127)
    return qlo, qhi


def _build_nc():
    import concourse.bass as bass
    import concourse.bacc as bacc
    import concourse.mybir as mybir
    from concourse import tile

    dt = mybir.dt
    bf = dt.bfloat16
    f32 = dt.float32
    AF = mybir.ActivationFunctionType

    nc = bacc.Bacc(
        "TRN2",
        target_bir_lowering=False,
        debug=False,
        enable_asserts=False,
        num_devices=NCORES,
    )

    # hidden, pre-tiled on host as [tb, p, a, t]: per token-block each SBUF
    # partition's data (all 32 a-chunks) is one contiguous 32KB DRAM run, so
    # the loads stream at full rate with a plain 2D access pattern.
    hiddenT = nc.dram_tensor("hiddenT", [NBLK * 128, NA * QB], bf,
                             kind="ExternalInput")
    # weights/masks host-prearranged to [128, chunks*width] matching the SBUF
    # tile layout exactly -> plain contiguous 2D DMAs
    wq = nc.dram_tensor("wq", [128, NA * HPC * DH], bf, kind="ExternalInput")
    wk = nc.dram_tensor("wk", [128, NA * DH], bf, kind="ExternalInput")
    wv = nc.dram_tensor("wv", [128, NA * DH], bf, kind="ExternalInput")
    wo = nc.dram_tensor("wo", [128, NA * HPC * DH], bf, kind="ExternalInput")
    cost = nc.dram_tensor("cost", [DH, S], bf, kind="ExternalInput")
    sinm = nc.dram_tensor("sinm", [DH, S], bf, kind="ExternalInput")
    maskt = nc.dram_tensor("maskt", [128, len(MASK_RS) * QB], bf, kind="ExternalInput")
    outT = nc.dram_tensor("outT", [HPC * DH, BT], f32, kind="ExternalOutput")

    with tile.TileContext(nc) as tc:
        with (
            tc.tile_pool(name="const", bufs=1) as cpool,
            tc.tile_pool(name="hid", bufs=8) as hidpool,
            tc.tile_pool(name="kv", bufs=2) as kvpool,
            tc.tile_pool(name="qt", bufs=8) as qtpool,
            tc.tile_pool(name="work", bufs=2) as wpool,
            tc.tile_pool(name="pt", bufs=4) as ptpool,
            tc.tile_pool(name="mm", bufs=4, space="PSUM") as mmpool,
            tc.tile_pool(name="sps", bufs=2, space="PSUM") as spool,
            tc.tile_pool(name="ctxps", bufs=2, space="PSUM") as cxpool,
            tc.tile_pool(name="dram", bufs=1, space="DRAM") as dpool,
        ):
            # ---- resident constants ----
            wq_sbs = [
                cpool.tile([128, w * HPC * DH], bf, tag=f"wq{i}", name=f"wq{i}")
                for i, (a0, w) in enumerate(SEG_FIRST)
            ]
            _wq_seg_of = {}
            for i, (a0, w) in enumerate(SEG_FIRST):
                for a in range(a0, a0 + w):
                    _wq_seg_of[a] = (i, a - a0)

            def wq_slice(a, h):
                i, ao = _wq_seg_of[a]
                return wq_sbs[i][:, ao * 512 + h * 128 : ao * 512 + (h + 1) * 128]

            wk_sb = cpool.tile([128, NA * DH], bf, tag="wk")
            wv_sb = cpool.tile([128, NA * DH], bf, tag="wv")
            wo_sb = cpool.tile([128, NA * HPC * DH], bf, tag="wo")
            cos_sb = cpool.tile([128, S], bf, tag="cos")
            sin_sb = cpool.tile([128, S], bf, tag="sin")
            mask_sb = cpool.tile([128, len(MASK_RS) * QB], bf, tag="mask")
            ones_sb = cpool.tile([128, 1], bf, tag="ones")

            # Startup preloads: few big DMAs, consumption-ordered. wo (4MB,
            # first needed ~200us in) is deferred to block 1 so it doesn't
            # compete for queue bandwidth during the DMA-bound first block.
            def wq_dma(eng, i):
                a0, w = SEG_FIRST[i]
                eng.dma_start(wq_sbs[i][:], wq[:, a0 * 512 : (a0 + w) * 512])

            wq_dma(nc.scalar, 0)
            wq_dma(nc.scalar, 1)
            nc.scalar.dma_start(cos_sb[:], cost[:])
            nc.scalar.dma_start(sin_sb[:], sinm[:])
            wq_dma(nc.gpsimd, 2)
            wq_dma(nc.gpsimd, 3)
            wq_dma(nc.gpsimd, 4)
            nc.gpsimd.dma_start(wk_sb[:], wk[:])
            nc.gpsimd.dma_start(wv_sb[:], wv[:])
            nc.gpsimd.dma_start(mask_sb[:], maskt[:])
            nc.any.memset(ones_sb[:], 1.0)

            def load_wo():
                for i in range(4):
                    nc.gpsimd.dma_start(
                        wo_sb[:, i * 4096 : (i + 1) * 4096],
                        wo[:, i * 4096 : (i + 1) * 4096],
                    )

            # per token-block collective buffers; block 7's is split by head
            # pair so its first AllGather can start mid-attention (the tail
            # o_proj would otherwise wait on the full last AllGather)
            ctxl = [
                dpool.tile([HPC * DH, QB], bf, tag=f"ctxl{k}", name=f"ctxl{k}")
                for k in range(NBLK - 1)
            ]
            ctxf = [
                dpool.tile([H * DH, QB], bf, addr_space="Shared",
                           tag=f"ctxf{k}", name=f"ctxf{k}")
                for k in range(NBLK - 1)
            ]
            ctxl7 = [
                dpool.tile([2 * DH, QB], bf, tag=f"ctxl7{i}", name=f"ctxl7{i}")
                for i in range(2)
            ]
            ctxf7 = [
                dpool.tile([NCORES * 2 * DH, QB], bf, addr_space="Shared",
                           tag=f"ctxf7{i}", name=f"ctxf7{i}")
                for i in range(2)
            ]
            # dummy collective to absorb the ~12us first-collective ramp
            # before the first real AllGather needs it
            duml = dpool.tile([8, 16], bf, tag="duml", name="duml")
            dumf = dpool.tile([64, 16], bf, addr_space="Shared", tag="dumf",
                              name="dumf")
            dums = cpool.tile([8, 16], bf, tag="dums")
            nc.any.memset(dums[:], 0.0)
            nc.gpsimd.dma_start(duml[:], dums[:])
            nc.gpsimd.collective_compute(
                "AllGather",
                mybir.AluOpType.bypass,
                replica_groups=[list(range(NCORES))],
                ins=[duml[:].opt()],
                outs=[dumf[:].opt()],
            )

            def load_hid_segs(k):
                segs = SEG_FIRST if k == 0 else SEG_COARSE
                out = []
                for a0, w in segs:
                    t = hidpool.tile([128, w * QB], bf, tag="hid", name=f"hid{k}_{a0}")
                    nc.sync.dma_start(
                        t[:],
                        hiddenT[k * 128 : (k + 1) * 128,
                                a0 * QB : (a0 + w) * QB],
                    )
                    out.append((a0, w, t))
                return out

            def rope_drain(ps, dst, qbi):
                """dst(bf16) = ps * cos + rot_half(ps) * sin (sign-folded)."""
                t1 = wpool.tile([128, QB], f32, tag="ropet1", bufs=1)
                t2 = wpool.tile([128, QB], f32, tag="ropet2", bufs=1)
                tok0 = qbi * QB
                cs = cos_sb[:, tok0 : tok0 + QB]
                sn = sin_sb[:, tok0 : tok0 + QB]
                nc.vector.tensor_mul(t1[:], ps, cs)
                nc.vector.tensor_mul(t2[0:64, :], ps[64:128, :], sn[0:64, :])
                nc.vector.tensor_mul(t2[64:128, :], ps[0:64, :], sn[64:128, :])
                nc.vector.tensor_add(dst, t1[:], t2[:])

            def v_fillers(k, qbi, v_sb, hsegs):
                """Direct v[tok, dh] projection for block k as filler pieces:
                hidden chunk as stationary, wv as moving. Each 128-token
                subtile accumulates in its own PSUM tile."""
                ltok = qbi * QB
                hs_of = {}
                for a0, w, hs in hsegs:
                    for ai in range(w):
                        hs_of[a0 + ai] = hs[:, ai * QB : (ai + 1) * QB]
                out = []
                state = {}

                def mk_chunk(j, a0):
                    def emit():
                        if j not in state:
                            state[j] = mmpool.tile(
                                [128, 128], f32, tag="mmps", name=f"vps{k}_{j}"
                            )
                        vps = state[j]
                        for a in range(a0, a0 + 8):
                            nc.tensor.matmul(
                                vps[:],
                                hs_of[a][:, j * 128 : (j + 1) * 128],
                                wv_sb[:, a * 128 : (a + 1) * 128],
                                start=(a == 0), stop=(a == NA - 1),
                            )
                    return emit

                def mk_drain(j):
                    def emit():
                        vps = state.pop(j)
                        nc.vector.tensor_copy(
                            v_sb[:, ltok + j * 128 : ltok + (j + 1) * 128], vps[:]
                        )
                    return emit

                # seg-major order: all four token-subtiles consume hid seg s
                # before any touches seg s+1, so fillers can start while the
                # later segments are still streaming in
                for a0 in range(0, NA, 8):
                    for j in range(4):
                        out.append(mk_chunk(j, a0))
                for j in range(4):
                    out.append(mk_drain(j))
                return out

            def proj_block(k, b, qbi, kT_sb, v_sb, hsegs, with_v):
                """q/k projections + RoPE (+ inline direct-v for early blocks)."""
                ltok = qbi * QB
                qts = [
                    qtpool.tile([128, QB], bf, tag="qtile", name=f"qt{k}_{h}")
                    for h in range(HPC)
                ]
                for grp in (("q0", "q1", "q2"), ("q3", "k")):
                    ps = {u: mmpool.tile([128, QB], f32, tag="mmps", name=f"ps{u}{k}")
                          for u in grp}
                    for a0, w, hs in hsegs:
                        for u in grp:
                            for ai in range(w):
                                a = a0 + ai
                                st = a == 0
                                sp = a == NA - 1
                                hcol = hs[:, ai * QB : (ai + 1) * QB]
                                if u[0] == "q":
                                    h = int(u[1])
                                    nc.tensor.matmul(
                                        ps[u][:], wq_slice(a, h), hcol,
                                        start=st, stop=sp,
                                    )
                                else:
                                    nc.tensor.matmul(
                                        ps[u][:],
                                        wk_sb[:, a * 128 : (a + 1) * 128],
                                        hcol,
                                        start=st, stop=sp,
                                    )
                    for u in grp:
                        if u[0] == "q":
                            rope_drain(ps[u][:], qts[int(u[1])][:], qbi)
                        else:
                            rope_drain(ps[u][:], kT_sb[:, ltok : ltok + QB], qbi)
                if with_v:
                    for f in v_fillers(k, qbi, v_sb, hsegs):
                        f()
                return qts

            class OProjChain:
                """o_proj for ctx block `opk`, emitted as small filler pieces."""

                def __init__(self, opk):
                    self.opk = opk
                    self.cfs = None

                def prefetch(self):
                    self.cfs = []
                    if self.opk < NBLK - 1:
                        src3 = ctxf[self.opk][:].rearrange("(a p) t -> p a t", p=128)
                        for qt4 in range(4):
                            t = hidpool.tile(
                                [128, NQTR * QB], bf, tag="hid",
                                name=f"cf{self.opk}_{qt4}",
                            )
                            nc.gpsimd.dma_start(
                                t[:].rearrange("p (a t) -> p a t", a=NQTR),
                                src3[:, qt4 * NQTR : (qt4 + 1) * NQTR, :],
                            )
                            self.cfs.append(t)
                    else:
                        # block 7's ctx is AllGathered in two head-pair halves:
                        # global chunk a = 4c + q lives in half q//2 at rows
                        # c*256 + (q%2)*128
                        srcs = [
                            ctxf7[i][:].rearrange("(cq p) t -> p cq t", p=128)
                            for i in range(2)
                        ]
                        for qt4 in range(4):
                            t = hidpool.tile(
                                [128, NQTR * QB], bf, tag="hid",
                                name=f"cf{self.opk}_{qt4}",
                            )
                            dst3 = t[:].rearrange("p (a t) -> p a t", a=NQTR)
                            for cr in range(2):
                                c = qt4 * 2 + cr
                                for i in range(2):
                                    nc.gpsimd.dma_start(
                                        dst3[:, cr * 4 + i * 2 : cr * 4 + i * 2 + 2, :],
                                        srcs[i][:, c * 2 : c * 2 + 2, :],
                                    )
                            self.cfs.append(t)

                def fillers(self):
                    out = []
                    state = {}

                    def mk_chunk(oc, a0):
                        def emit():
                            if oc not in state:
                                state[oc] = mmpool.tile(
                                    [128, QB], f32, tag="mmps",
                                    name=f"ops{self.opk}_{oc}",
                                )
                            ps = state[oc]
                            for a in range(a0, a0 + 4):
                                nc.tensor.matmul(
                                    ps[:],
                                    wo_sb[:, a * 512 + oc * 128 : a * 512 + (oc + 1) * 128],
                                    self.cfs[a // NQTR][:, (a % NQTR) * QB : (a % NQTR + 1) * QB],
                                    start=(a == 0), stop=(a == NA - 1),
                                )
                        return emit

                    def mk_drain(oc):
                        def emit():
                            ps = state.pop(oc)
                            osb = wpool.tile([128, QB], f32, tag="osb",
                                             name=f"ob{self.opk}_{oc}", bufs=1)
                            nc.vector.tensor_copy(osb[:], ps[:])
                            nc.sync.dma_start(
                                outT[oc * 128 : (oc + 1) * 128,
                                     self.opk * QB : (self.opk + 1) * QB],
                                osb[:],
                            )
                        return emit

                    for oc in range(HPC):
                        for a0 in range(0, NA, 4):
                            out.append(mk_chunk(oc, a0))
                        out.append(mk_drain(oc))
                    return out

            def attn_block(k, b, qbi, qts, kT_sb, v_sb, fillers, skip_slots=0):
                Q0 = 4 * qbi
                kts = [Q0] + [kt for kt in range(max(0, Q0 - 8), Q0 + 4) if kt != Q0]
                nslots = HPC * (len(kts) + 1)
                eff = max(1, nslots - skip_slots)
                nf = len(fillers)
                fi = 0
                slot = 0

                def tick():
                    nonlocal fi, slot
                    slot += 1
                    if slot <= skip_slots:
                        return
                    want = nf * (slot - skip_slots) // eff
                    while fi < want:
                        fillers[fi]()
                        fi += 1

                for h in range(HPC):
                    qt = qts[h]
                    ctx_ps = cxpool.tile([128, QB], f32, tag="ctxps", name=f"cx{k}{h}")
                    l_ps = spool.tile([1, QB], f32, tag="sps", name=f"l{k}{h}")
                    # bf16 accumulator for the prob-sum: 2x DVE rate and feeds
                    # the partition-sum ones-matmul directly
                    acc = wpool.tile([128, QB], bf, tag="lacc", name=f"la{k}{h}")
                    for idx, kt in enumerate(kts):
                        r = kt - (Q0 - 8)
                        qlo, qhi = _span(r)
                        s_ps = spool.tile([128, QB], f32, tag="sps", name=f"s{k}{h}{kt}")
                        nc.tensor.matmul(
                            s_ps[:, qlo:qhi],
                            kT_sb[:, kt * 128 : (kt + 1) * 128],
                            qt[:, qlo:qhi],
                            start=True, stop=True,
                        )
                        pt = ptpool.tile([128, QB], bf, tag="pt", name=f"pt{k}{h}{kt}")
                        nc.scalar.activation(
                            pt[:, qlo:qhi], s_ps[:, qlo:qhi], AF.Exp, scale=SCALE
                        )
                        if r in MSLOT:
                            m0 = MSLOT[r] * QB
                            nc.vector.tensor_mul(
                                pt[:, qlo:qhi],
                                pt[:, qlo:qhi],
                                mask_sb[:, m0 + qlo : m0 + qhi],
                            )
                        last = idx == len(kts) - 1
                        nc.tensor.matmul(
                            ctx_ps[:, qlo:qhi],
                            v_sb[:, kt * 128 : (kt + 1) * 128],
                            pt[:, qlo:qhi],
                            start=(idx == 0), stop=last,
                        )
                        if idx == 0:
                            nc.vector.tensor_copy(acc[:], pt[:])
                        else:
                            nc.vector.tensor_add(
                                acc[:, qlo:qhi], acc[:, qlo:qhi], pt[:, qlo:qhi]
                            )
                        tick()
                    nc.tensor.matmul(
                        l_ps[0:1, :], ones_sb[:, 0:1], acc[:], start=True, stop=True
                    )
                    lrec = wpool.tile([1, QB], f32, tag="lrec", name=f"lr{k}{h}", bufs=1)
                    nc.vector.reciprocal_approx_fast(lrec[:], l_ps[:])
                    lb = wpool.tile([128, QB], f32, tag="lb", name=f"lb{k}{h}")
                    nc.gpsimd.partition_broadcast(lb[:], lrec[0:1, :])
                    ctx_sb = wpool.tile([128, QB], bf, tag="ctxsb", name=f"cs{k}{h}")
                    nc.vector.tensor_mul(ctx_sb[:], ctx_ps[:], lb[:])
                    if k == NBLK - 1:
                        nc.sync.dma_start(
                            ctxl7[h // 2][(h % 2) * 128 : (h % 2 + 1) * 128, :],
                            ctx_sb[:],
                        )
                        if h % 2 == 1:
                            _ag(ctxl7[h // 2], ctxf7[h // 2])
                    else:
                        nc.sync.dma_start(
                            ctxl[k][h * 128 : (h + 1) * 128, :], ctx_sb[:]
                        )
                    tick()
                while fi < nf:
                    fillers[fi]()
                    fi += 1

            def _ag(lt, ft):
                nc.gpsimd.collective_compute(
                    "AllGather",
                    mybir.AluOpType.bypass,
                    replica_groups=[list(range(NCORES))],
                    ins=[lt[:].opt()],
                    outs=[ft[:].opt()],
                )

            def allgather(k):
                _ag(ctxl[k], ctxf[k])

            # ================= emission schedule =================
            chains = [OProjChain(opk) for opk in range(NBLK)]
            kT = {}
            vS = {}
            for b in range(B):
                kT[b] = kvpool.tile([128, S], bf, tag="kT", name=f"kT{b}")
                vS[b] = kvpool.tile([128, S], bf, tag="v", name=f"v{b}")

            hseg_of = {0: load_hid_segs(0)}
            for k in range(NBLK):
                b, qbi = divmod(k, NQB)
                # from block 2 on, prefetch next hidden at block top so the
                # next block's v-projection can be woven into this attention
                if k >= 2 and k + 1 < NBLK:
                    hseg_of[k + 1] = load_hid_segs(k + 1)
                qts = proj_block(k, b, qbi, kT[b], vS[b], hseg_of[k],
                                 with_v=(k <= 1))
                hseg_of.pop(k)
                if k == 1:
                    load_wo()
                # cf prefetch: the DMA only fires once proj(k) releases its
                # hid-pool slots, so cf(k-2) is ready one block later — the
                # weave map below lags by 3 blocks to match. cf5/cf6 are NOT
                # prefetched at block-7 top: their slot-WAR-blocked triggers
                # would sit at the head of the gpsimd queue and stall the
                # attention-7 epilogue broadcasts (delaying the last
                # AllGather by ~15us).
                if 2 <= k <= 6:
                    chains[k - 2].prefetch()
                if k == 7:
                    chains[6].prefetch()
                # early blocks: next hidden prefetched here instead — ahead of
                # this block's ctx stores in the sync queue, behind startup
                if k < 2:
                    hseg_of[k + 1] = load_hid_segs(k + 1)
                fillers = []
                if 3 <= k <= 6:
                    fillers += chains[k - 3].fillers()
                elif k == 7:
                    fillers += chains[4].fillers()
                if 1 <= k <= 6:
                    # v of the next block last: its hidden is still streaming
                    # in during this attention phase
                    nb, nq = divmod(k + 1, NQB)
                    fillers += v_fillers(k + 1, nq, vS[nb], hseg_of[k + 1])
                # at k=1 the v fillers are the whole list and hid(2) only
                # starts landing ~11us into the attention: delay the pacing
                attn_block(k, b, qbi, qts, kT[b], vS[b], fillers,
                           skip_slots=(16 if k == 1 else 0))
                if k < NBLK - 1:
                    allgather(k)
            # tail: op6 -> op5 -> op7. op6 (input long ready) covers cf5's
            # load; op5 covers the last AllGather + cf7 so op7's input is
            # ready by the time the PE reaches it.
            chains[5].prefetch()
            for f in chains[6].fillers():
                f()
            for f in chains[5].fillers():
                f()
            chains[7].prefetch()
            for f in chains[7].fillers():
                f()

    nc.compile()
    return nc


_NC = None


def _get_nc():
    global _NC
    if _NC is None:
        _NC = _build_nc()
    return _NC


def _prep_inputs(hidden_states, q_proj_w, k_proj_w, v_proj_w, o_proj_w, position_ids):
    hidden_states = np.asarray(hidden_states, dtype=np.float32)
    # pre-tile: hT[tb, p, a, t] = hidden[tb*QB + t, a*128 + p]
    hT = np.ascontiguousarray(
        hidden_states.reshape(BT // QB, QB, NA, 128).transpose(0, 3, 2, 1)
    ).astype(BF16).reshape((BT // QB) * 128, NA * QB)

    pos = np.asarray(position_ids)[0].astype(np.float32)  # [S]
    inv = 1.0 / (THETA ** (np.arange(0, DH, 2, dtype=np.float32) / DH))  # [64]
    ang = pos[:, None] * inv[None, :]  # [S, 64]
    c = np.cos(ang).T.astype(np.float32)  # [64, S]
    s = np.sin(ang).T.astype(np.float32)
    cost = np.ascontiguousarray(np.concatenate([c, c], axis=0)).astype(BF16)
    sinm = np.ascontiguousarray(np.concatenate([-s, s], axis=0)).astype(BF16)

    kj = np.arange(128)[:, None]
    qi = np.arange(QB)[None, :]
    masks = []
    for r in MASK_RS:
        d = (8 - r) * 128 + qi - kj
        masks.append(((d >= 0) & (d < WIN)).astype(np.float32))
    maskt = np.ascontiguousarray(np.concatenate(masks, axis=0)).astype(BF16)

    q_proj_w = np.asarray(q_proj_w, dtype=np.float32)
    k_proj_w = np.asarray(k_proj_w, dtype=np.float32)
    v_proj_w = np.asarray(v_proj_w, dtype=np.float32)
    o_proj_w = np.asarray(o_proj_w, dtype=np.float32)

    def wtile(wT):
        """[HID, D] (hid-major) -> [128, NA*D] matching SBUF layout:
        out[p, a*D+dd] = wT[a*128+p, dd]."""
        dcols = wT.shape[1]
        return np.ascontiguousarray(
            wT.reshape(NA, 128, dcols).transpose(1, 0, 2).reshape(128, NA * dcols)
        ).astype(BF16)

    # maskt: [128, m*QB] with slot m at cols [m*QB, (m+1)*QB)
    maskt = np.ascontiguousarray(
        maskt.reshape(len(MASK_RS), 128, QB).transpose(1, 0, 2).reshape(
            128, len(MASK_RS) * QB
        )
    )

    in_maps = []
    for core in range(NCORES):
        r0q = core * HPC * DH
        r0k = core * DH
        in_maps.append(
            {
                "hiddenT": hT,
                "wq": wtile(q_proj_w[r0q : r0q + HPC * DH, :].T),
                "wk": wtile(k_proj_w[r0k : r0k + DH, :].T),
                "wv": wtile(v_proj_w[r0k : r0k + DH, :].T),
                "wo": wtile(o_proj_w[r0q : r0q + HPC * DH, :].T),
                "cost": cost,
                "sinm": sinm,
                "maskt": maskt,
            }
        )
    return in_maps


def run(inputs, trace=False):
    from concourse.bass_utils import run_bass_kernel_spmd

    nc = _get_nc()
    in_maps = _prep_inputs(
        inputs["hidden_states"],
        inputs["q_proj_w"],
        inputs["k_proj_w"],
        inputs["v_proj_w"],
        inputs["o_proj_w"],
        inputs["position_ids"],
    )
    res = run_bass_kernel_spmd(
        nc, in_maps, core_ids=list(range(NCORES)), trace=trace
    )
    out = np.empty((BT, HID), dtype=np.float32)
    for core in range(NCORES):
        o = np.asarray(res.results[core]["outT"], dtype=np.float32)  # [512, BT]
        out[:, core * HPC * DH : (core + 1) * HPC * DH] = o.T
    return out.reshape(B, S, HID), res


def kernel(**inputs):
    out, _ = run(inputs, trace=False)
    return out
